# revision 18
# baseline (speedup 1.0000x reference)
"""Bidirectional GRU encoder (Keras reset_after semantics) on 8 trn2 NeuronCores.

Sharding: 4 core-pairs x 16 batch rows. Within a pair, core 2p runs the
forward GRU and core 2p+1 the backward GRU (as a forward scan over the
time-reversed sequence -- direction is pure input data, the program is
uniform SPMD). Each core: embedding gather -> gx = e@W + bias (PE, f32r)
-> 256-step recurrent scan (PE f32r matmuls + ACT sigmoid/tanh + DVE
elementwise, state kept transposed for the next step's stationary operand)
-> LayerNorm of its own 512-unit half using pairwise-AllReduced per-(b,t)
sum/sumsq stats. Host assembles halves/reverses time for bwd cores.
"""
import sys

sys.path.insert(0, "/opt/trn_rl_repo")

import numpy as np  # noqa: E402

import concourse.bass as bass  # noqa: E402
import concourse.bacc as bacc  # noqa: E402
import concourse.tile as tile  # noqa: E402
from concourse import mybir  # noqa: E402
from concourse.bass_utils import run_bass_kernel_spmd  # noqa: E402
from concourse.masks import make_identity  # noqa: E402

V, D, U, G = 32000, 512, 512, 1536  # vocab, embed dim, per-dir units, 3U
B, T, BL = 64, 256, 16  # full batch, seq len, batch per core
KD = D // 128  # 4 contraction tiles for D
KU = U // 128  # 4 contraction tiles for U
LN_EPS = 1e-3
F32 = mybir.dt.float32
F32R = mybir.dt.float32r
I32 = mybir.dt.int32
AF = mybir.ActivationFunctionType
OP = mybir.AluOpType

_cache = {}


def _build(t_steps=T):
    nc = bacc.Bacc(None, target_bir_lowering=False, debug=False)
    ntok = BL * t_steps
    nchunk = ntok // 128
    assert ntok % 128 == 0

    # --- external I/O (per core) ---
    x_d = nc.dram_tensor("x", [ntok, 1], I32, kind="ExternalInput").ap()
    emb_d = nc.dram_tensor("emb", [V, D], F32, kind="ExternalInput").ap()
    w_d = nc.dram_tensor("w", [D, G], F32R, kind="ExternalInput").ap()
    uw_d = nc.dram_tensor("uw", [U, G], F32R, kind="ExternalInput").ap()
    bias_d = nc.dram_tensor("bias", [128, G], F32, kind="ExternalInput").ap()
    gam_d = nc.dram_tensor("gam", [128, U], F32, kind="ExternalInput").ap()
    bet_d = nc.dram_tensor("bet", [128, U], F32, kind="ExternalInput").ap()
    scat_d = nc.dram_tensor("scat", [ntok, 1], I32, kind="ExternalInput").ap()
    outh_d = nc.dram_tensor("outh", [ntok, U], F32, kind="ExternalOutput").ap()
    state_d = nc.dram_tensor("state", [BL, U], F32, kind="ExternalOutput").ap()

    # --- internal DRAM ---
    gx_d = nc.dram_tensor("gx_i", [BL, t_steps, G], F32).ap()
    raw_d = nc.dram_tensor("raw_i", [BL, t_steps, U], F32).ap()
    stats_d = nc.dram_tensor("stats_i", [ntok, 2], F32).ap()
    statsr_d = nc.dram_tensor("statsr_i", [ntok, 2], F32).ap()

    def chunk_3d(d3, c, width):
        """flat rows [c*128,(c+1)*128) of a [BL, t_steps, width] tensor."""
        if t_steps >= 128:
            per_b = t_steps // 128
            b, t0 = c // per_b, (c % per_b) * 128
            return d3[b, t0 : t0 + 128, :]
        nb = 128 // t_steps
        b0 = c * nb
        return d3[b0 : b0 + nb, :, :]

    from contextlib import ExitStack

    with tile.TileContext(nc) as tc, ExitStack() as stack:
        cst = stack.enter_context(tc.tile_pool(name="cst", bufs=1))
        # constants
        wall = cst.tile([128, KD * G], F32R, tag="wall")
        uwall = cst.tile([128, KU * G], F32R, tag="uwall")
        for k in range(KD):
            nc.sync.dma_start(wall[:, k * G : (k + 1) * G], w_d[k * 128 : (k + 1) * 128, :])
            nc.sync.dma_start(uwall[:, k * G : (k + 1) * G], uw_d[k * 128 : (k + 1) * 128, :])
        bias_sb = cst.tile([128, G], F32, tag="bias")
        nc.sync.dma_start(bias_sb[:], bias_d[:])
        gam_sb = cst.tile([128, U], F32, tag="gam")
        nc.sync.dma_start(gam_sb[:], gam_d[:])
        bet_sb = cst.tile([128, U], F32, tag="bet")
        nc.sync.dma_start(bet_sb[:], bet_d[:])
        id_sb = cst.tile([128, 128], F32, tag="id")
        make_identity(nc, id_sb[:])
        # mask: m_all[b, t] = (x[b,t] != 0) as 1.0/0.0
        xt = cst.tile([BL, t_steps], I32, tag="xt")
        nc.sync.dma_start(xt[:], x_d[:, 0:1])
        xf = cst.tile([BL, t_steps], F32, tag="xf")
        nc.vector.tensor_copy(xf[:], xt[:])
        m_all = cst.tile([BL, t_steps], F32, tag="mall")
        nc.vector.tensor_scalar(m_all[:], xf[:], 0.0, None, OP.not_equal)

        # ---------------- stage A: gather + gx = e@W + bias ----------------
        with (
            tc.tile_pool(name="sa", bufs=3) as sa,
            tc.tile_pool(name="sa_ps", bufs=2, space="PSUM") as sa_ps,
        ):
            for c in range(nchunk):
                ix = sa.tile([128, 1], I32, tag="ix")
                nc.sync.dma_start(ix[:], x_d[c * 128 : (c + 1) * 128, :])
                et = sa.tile([128, D], F32, tag="et")
                nc.gpsimd.indirect_dma_start(
                    out=et[:], out_offset=None, in_=emb_d[:],
                    in_offset=bass.IndirectOffsetOnAxis(ap=ix[:, 0:1], axis=0),
                )
                tp = sa_ps.tile([128, D], F32, tag="tp")
                for k in range(KD):
                    nc.tensor.transpose(
                        tp[:, k * 128 : (k + 1) * 128],
                        et[:, k * 128 : (k + 1) * 128],
                        id_sb[:],
                    )
                etr = sa.tile([128, D], F32R, tag="etr")
                nc.vector.tensor_copy(etr[:], tp[:])
                ps = sa_ps.tile([128, G], F32, tag="ps")
                for g in range(3):
                    for k in range(KD):
                        nc.tensor.matmul(
                            ps[:, g * 512 : (g + 1) * 512],
                            etr[:, k * 128 : (k + 1) * 128],
                            wall[:, k * G + g * 512 : k * G + (g + 1) * 512],
                            start=(k == 0),
                            stop=(k == KD - 1),
                        )
                gxc = sa.tile([128, G], F32, tag="gxc")
                nc.vector.tensor_tensor(out=gxc[:], in0=ps[:], in1=bias_sb[:], op=OP.add)
                nc.sync.dma_start(chunk_3d(gx_d, c, G), gxc[:])

        # ---------------- stage B: recurrent scan ----------------
        with (
            tc.tile_pool(name="gxt", bufs=4) as gxp,
            tc.tile_pool(name="ew", bufs=2) as ew,
            tc.tile_pool(name="hp", bufs=2) as hp,
            tc.tile_pool(name="htp", bufs=2) as htp,
            tc.tile_pool(name="sc_ps", bufs=2, space="PSUM") as sc_ps,
            tc.tile_pool(name="tr_ps", bufs=2, space="PSUM") as tr_ps,
        ):
            h_prev = hp.tile([BL, U], F32, tag="h")
            nc.vector.memset(h_prev[:], 0.0)
            ht_prev = htp.tile([128, KU * BL], F32R, tag="ht")
            z0 = hp.tile([128, KU * BL], F32, tag="z0")
            nc.vector.memset(z0[:], 0.0)
            nc.vector.tensor_copy(ht_prev[:], z0[:])
            for t in range(t_steps):
                gxt = gxp.tile([BL, G], F32, tag="gxt")
                nc.sync.dma_start(gxt[:], gx_d[:, t, :])
                ps = sc_ps.tile([BL, G], F32, tag="ps")
                for g in range(3):
                    for k in range(KU):
                        nc.tensor.matmul(
                            ps[:, g * 512 : (g + 1) * 512],
                            ht_prev[:, k * BL : (k + 1) * BL],
                            uwall[:, k * G + g * 512 : k * G + (g + 1) * 512],
                            start=(k == 0),
                            stop=(k == KU - 1),
                        )
                zr = ew.tile([BL, 2 * 512], F32, tag="zr")
                nc.vector.tensor_tensor(out=zr[:], in0=ps[:, 0:1024], in1=gxt[:, 0:1024], op=OP.add)
                zn = ew.tile([BL, 512], F32, tag="zn")  # 1 - z
                nc.scalar.activation(zn[:], zr[:, 0:512], AF.Sigmoid, scale=-1.0)
                rr = ew.tile([BL, 512], F32, tag="rr")
                nc.scalar.activation(rr[:], zr[:, 512:1024], AF.Sigmoid)
                zm = ew.tile([BL, 512], F32, tag="zm")  # (1-z)*mask
                nc.vector.tensor_scalar_mul(zm[:], zn[:], m_all[:, t : t + 1])
                tmp = ew.tile([BL, 512], F32, tag="tmp")  # r * hh
                nc.vector.tensor_tensor(out=tmp[:], in0=rr[:], in1=ps[:, 1024:1536], op=OP.mult)
                hca = ew.tile([BL, 512], F32, tag="hca")
                nc.vector.tensor_tensor(out=hca[:], in0=tmp[:], in1=gxt[:, 1024:1536], op=OP.add)
                hc = ew.tile([BL, 512], F32, tag="hc")
                nc.scalar.activation(hc[:], hca[:], AF.Tanh)
                e1 = ew.tile([BL, 512], F32, tag="e1")
                nc.vector.tensor_tensor(out=e1[:], in0=hc[:], in1=h_prev[:], op=OP.subtract)
                f1 = ew.tile([BL, 512], F32, tag="f1")
                nc.vector.tensor_tensor(out=f1[:], in0=e1[:], in1=zm[:], op=OP.mult)
                h_new = hp.tile([BL, U], F32, tag="h")
                nc.vector.tensor_tensor(out=h_new[:], in0=h_prev[:], in1=f1[:], op=OP.add)
                nc.sync.dma_start(raw_d[:, t, :], h_new[:])
                tps = tr_ps.tile([128, KU * BL], F32, tag="tps")
                for k in range(KU):
                    nc.tensor.transpose(
                        tps[:, k * BL : (k + 1) * BL],
                        h_new[:, k * 128 : (k + 1) * 128],
                        id_sb[:BL, :BL],
                    )
                ht_new = htp.tile([128, KU * BL], F32R, tag="ht")
                nc.vector.tensor_copy(ht_new[:], tps[:])
                h_prev, ht_prev = h_new, ht_new
            # state output = h at last step
            nc.sync.dma_start(state_d[:], h_prev[:])

        # ---------------- stage C: LayerNorm ----------------
        with tc.tile_pool(name="sc", bufs=3) as sc:
            for c in range(nchunk):
                rawc = sc.tile([128, U], F32, tag="rawc")
                nc.sync.dma_start(rawc[:], chunk_3d(raw_d, c, U))
                s1 = sc.tile([128, 1], F32, tag="s1")
                nc.vector.reduce_sum(out=s1[:], in_=rawc[:], axis=mybir.AxisListType.X)
                sq = sc.tile([128, U], F32, tag="sq")
                nc.scalar.activation(sq[:], rawc[:], AF.Square)
                st = sc.tile([128, 2], F32, tag="st")
                nc.vector.reduce_sum(out=st[:, 1:2], in_=sq[:], axis=mybir.AxisListType.X)
                nc.vector.tensor_copy(st[:, 0:1], s1[:])
                ixc = sc.tile([128, 1], I32, tag="ixc")
                nc.sync.dma_start(ixc[:], scat_d[c * 128 : (c + 1) * 128, :])
                nc.gpsimd.indirect_dma_start(
                    out=stats_d[:],
                    out_offset=bass.IndirectOffsetOnAxis(ap=ixc[:, 0:1], axis=0),
                    in_=st[:], in_offset=None,
                )
            nc.gpsimd.collective_compute(
                "AllReduce", OP.add,
                replica_groups=[[0, 1], [2, 3], [4, 5], [6, 7]],
                ins=[stats_d[:]], outs=[statsr_d[:]],
            )
            for c in range(nchunk):
                ixc = sc.tile([128, 1], I32, tag="ixc")
                nc.sync.dma_start(ixc[:], scat_d[c * 128 : (c + 1) * 128, :])
                cstt = sc.tile([128, 2], F32, tag="cstt")
                nc.gpsimd.indirect_dma_start(
                    out=cstt[:], out_offset=None, in_=statsr_d[:],
                    in_offset=bass.IndirectOffsetOnAxis(ap=ixc[:, 0:1], axis=0),
                )
                mu = sc.tile([128, 1], F32, tag="mu")
                nc.scalar.mul(mu[:], cstt[:, 0:1], 1.0 / (2 * U))
                mq = sc.tile([128, 1], F32, tag="mq")
                nc.scalar.mul(mq[:], cstt[:, 1:2], 1.0 / (2 * U))
                mu2 = sc.tile([128, 1], F32, tag="mu2")
                nc.vector.tensor_tensor(out=mu2[:], in0=mu[:], in1=mu[:], op=OP.mult)
                var = sc.tile([128, 1], F32, tag="var")
                nc.vector.tensor_tensor(out=var[:], in0=mq[:], in1=mu2[:], op=OP.subtract)
                vare = sc.tile([128, 1], F32, tag="vare")
                nc.vector.tensor_scalar_add(vare[:], var[:], LN_EPS)
                rec = sc.tile([128, 1], F32, tag="rec")
                nc.vector.reciprocal(rec[:], vare[:])
                rstd = sc.tile([128, 1], F32, tag="rstd")
                nc.scalar.activation(rstd[:], rec[:], AF.Sqrt)
                rawc = sc.tile([128, U], F32, tag="rawc")
                nc.sync.dma_start(rawc[:], chunk_3d(raw_d, c, U))
                y = sc.tile([128, U], F32, tag="y")
                nc.vector.tensor_scalar(y[:], rawc[:], mu[:, 0:1], rstd[:, 0:1], OP.subtract, OP.mult)
                yg = sc.tile([128, U], F32, tag="yg")
                nc.vector.tensor_tensor(out=yg[:], in0=y[:], in1=gam_sb[:], op=OP.mult)
                yb = sc.tile([128, U], F32, tag="yb")
                nc.vector.tensor_tensor(out=yb[:], in0=yg[:], in1=bet_sb[:], op=OP.add)
                nc.sync.dma_start(outh_d[c * 128 : (c + 1) * 128, :], yb[:])
    nc.compile()
    return nc


def _prep_inputs(x, emb, W_fwd, U_fwd, b_fwd, W_bwd, U_bwd, b_bwd, gamma, beta, t_steps=T):
    f32 = np.float32
    in_maps = []
    for core in range(8):
        p, d = core // 2, core % 2
        xb = np.asarray(x[p * BL : (p + 1) * BL, :t_steps], dtype=np.int32)
        if d == 1:
            xb = xb[:, ::-1]
        Wd, Ud, bd = (W_fwd, U_fwd, b_fwd) if d == 0 else (W_bwd, U_bwd, b_bwd)
        bias = np.asarray(bd[0], f32).copy()
        bias[:1024] += np.asarray(bd[1][:1024], f32)
        gh = np.asarray(gamma[d * U : (d + 1) * U], f32)
        bh = np.asarray(beta[d * U : (d + 1) * U], f32)
        tloc = np.arange(t_steps)
        tabs = tloc if d == 0 else (t_steps - 1 - tloc)
        scat = (np.arange(BL)[:, None] * t_steps + tabs[None, :]).reshape(-1, 1)
        in_maps.append({
            "x": np.ascontiguousarray(xb).reshape(-1, 1),
            "emb": np.asarray(emb, f32),
            "w": np.asarray(Wd, f32),
            "uw": np.asarray(Ud, f32),
            "bias": np.tile(bias[None, :], (128, 1)),
            "gam": np.tile(gh[None, :], (128, 1)),
            "bet": np.tile(bh[None, :], (128, 1)),
            "scat": scat.astype(np.int32),
        })
    return in_maps


def _assemble(results, t_steps=T):
    out = np.empty((B, t_steps, 2 * U), np.float32)
    state = np.empty((B, 2 * U), np.float32)
    for p in range(4):
        rf, rb = results[2 * p], results[2 * p + 1]
        sl = slice(p * BL, (p + 1) * BL)
        out[sl, :, :U] = rf["outh"].reshape(BL, t_steps, U)
        out[sl, :, U:] = rb["outh"].reshape(BL, t_steps, U)[:, ::-1, :]
        state[sl, :U] = rf["state"]
        state[sl, U:] = rb["state"]
    return out, state


def run(inputs, t_steps=T):
    if t_steps not in _cache:
        _cache[t_steps] = _build(t_steps)
    nc = _cache[t_steps]
    in_maps = _prep_inputs(**inputs, t_steps=t_steps)
    res = run_bass_kernel_spmd(nc, in_maps, list(range(8)))
    return _assemble(res.results, t_steps)


def kernel(x, emb, W_fwd, U_fwd, b_fwd, W_bwd, U_bwd, b_bwd, gamma, beta):
    out, state = run(dict(x=x, emb=emb, W_fwd=W_fwd, U_fwd=U_fwd, b_fwd=b_fwd,
                          W_bwd=W_bwd, U_bwd=U_bwd, b_bwd=b_bwd,
                          gamma=gamma, beta=beta))
    return out, state


# revision 20
# speedup vs baseline: 1.2286x; 1.2286x over previous
"""Bidirectional GRU encoder (Keras reset_after semantics) on 8 trn2 NeuronCores.

Sharding: 4 core-pairs x 16 batch rows. Within a pair, core 2p runs the
forward GRU and core 2p+1 the backward GRU (as a forward scan over the
time-reversed sequence -- direction is pure input data, the program is
uniform SPMD). Each core: embedding gather -> gx = e@W + bias (PE, f32r)
-> 256-step recurrent scan (PE f32r matmuls + ACT sigmoid/tanh + DVE
elementwise, state kept transposed for the next step's stationary operand)
-> LayerNorm of its own 512-unit half using pairwise-AllReduced per-(b,t)
sum/sumsq stats. Host assembles halves/reverses time for bwd cores.
"""
import sys

sys.path.insert(0, "/opt/trn_rl_repo")

import numpy as np  # noqa: E402

import concourse.bass as bass  # noqa: E402
import concourse.bacc as bacc  # noqa: E402
import concourse.tile as tile  # noqa: E402
from concourse import mybir  # noqa: E402
from concourse.bass_utils import run_bass_kernel_spmd  # noqa: E402
from concourse.masks import make_identity  # noqa: E402

V, D, U, G = 32000, 512, 512, 1536  # vocab, embed dim, per-dir units, 3U
B, T, BL = 64, 256, 16  # full batch, seq len, batch per core
KD = D // 128  # 4 contraction tiles for D
KU = U // 128  # 4 contraction tiles for U
LN_EPS = 1e-3
F32 = mybir.dt.float32
F32R = mybir.dt.float32r
I32 = mybir.dt.int32
AF = mybir.ActivationFunctionType
OP = mybir.AluOpType

_cache = {}


def _build(t_steps=T):
    nc = bacc.Bacc(None, target_bir_lowering=False, debug=False)
    ntok = BL * t_steps
    nchunk = ntok // 128
    assert ntok % 128 == 0

    # --- external I/O (per core) ---
    x_d = nc.dram_tensor("x", [ntok, 1], I32, kind="ExternalInput").ap()
    emb_d = nc.dram_tensor("emb", [V, D], F32, kind="ExternalInput").ap()
    w_d = nc.dram_tensor("w", [D, G], F32R, kind="ExternalInput").ap()
    uw_d = nc.dram_tensor("uw", [U, G], F32R, kind="ExternalInput").ap()
    bias_d = nc.dram_tensor("bias", [128, G], F32, kind="ExternalInput").ap()
    gam_d = nc.dram_tensor("gam", [128, U], F32, kind="ExternalInput").ap()
    bet_d = nc.dram_tensor("bet", [128, U], F32, kind="ExternalInput").ap()
    scat_d = nc.dram_tensor("scat", [ntok, 1], I32, kind="ExternalInput").ap()
    outh_d = nc.dram_tensor("outh", [ntok, U], F32, kind="ExternalOutput").ap()
    state_d = nc.dram_tensor("state", [BL, U], F32, kind="ExternalOutput").ap()

    # --- internal DRAM ---
    gx_d = nc.dram_tensor("gx_i", [BL, t_steps, G], F32).ap()
    raw_d = nc.dram_tensor("raw_i", [BL, t_steps, U], F32).ap()
    stats_d = nc.dram_tensor("stats_i", [ntok, 2], F32).ap()
    statsr_d = nc.dram_tensor("statsr_i", [ntok, 2], F32).ap()

    def chunk_3d(d3, c, width):
        """flat rows [c*128,(c+1)*128) of a [BL, t_steps, width] tensor."""
        if t_steps >= 128:
            per_b = t_steps // 128
            b, t0 = c // per_b, (c % per_b) * 128
            return d3[b, t0 : t0 + 128, :]
        nb = 128 // t_steps
        b0 = c * nb
        return d3[b0 : b0 + nb, :, :]

    from contextlib import ExitStack

    with tile.TileContext(nc) as tc, ExitStack() as stack:
        cst = stack.enter_context(tc.tile_pool(name="cst", bufs=1))
        # constants
        wall = cst.tile([128, KD * G], F32R, tag="wall")
        uwall = cst.tile([128, KU * G], F32R, tag="uwall")
        for k in range(KD):
            nc.sync.dma_start(wall[:, k * G : (k + 1) * G], w_d[k * 128 : (k + 1) * 128, :])
            nc.sync.dma_start(uwall[:, k * G : (k + 1) * G], uw_d[k * 128 : (k + 1) * 128, :])
        bias_sb = cst.tile([128, G], F32, tag="bias")
        nc.sync.dma_start(bias_sb[:], bias_d[:])
        gam_sb = cst.tile([128, U], F32, tag="gam")
        nc.sync.dma_start(gam_sb[:], gam_d[:])
        bet_sb = cst.tile([128, U], F32, tag="bet")
        nc.sync.dma_start(bet_sb[:], bet_d[:])
        id_sb = cst.tile([128, 128], F32, tag="id")
        make_identity(nc, id_sb[:])
        # mask: m_all[b, t] = (x[b,t] != 0) as 1.0/0.0
        xt = cst.tile([BL, t_steps], I32, tag="xt")
        nc.sync.dma_start(xt[:], x_d[:, 0:1])
        xf = cst.tile([BL, t_steps], F32, tag="xf")
        nc.vector.tensor_copy(xf[:], xt[:])
        m_all = cst.tile([BL, t_steps], F32, tag="mall")
        nc.vector.tensor_scalar(m_all[:], xf[:], 0.0, None, OP.not_equal)

        # ---------------- stage A: gather + gx = e@W + bias ----------------
        with (
            tc.tile_pool(name="sa", bufs=3) as sa,
            tc.tile_pool(name="sa_ps", bufs=2, space="PSUM") as sa_ps,
        ):
            for c in range(nchunk):
                ix = sa.tile([128, 1], I32, tag="ix")
                nc.sync.dma_start(ix[:], x_d[c * 128 : (c + 1) * 128, :])
                et = sa.tile([128, D], F32, tag="et")
                nc.gpsimd.indirect_dma_start(
                    out=et[:], out_offset=None, in_=emb_d[:],
                    in_offset=bass.IndirectOffsetOnAxis(ap=ix[:, 0:1], axis=0),
                )
                tp = sa_ps.tile([128, D], F32, tag="tp")
                for k in range(KD):
                    nc.tensor.transpose(
                        tp[:, k * 128 : (k + 1) * 128],
                        et[:, k * 128 : (k + 1) * 128],
                        id_sb[:],
                    )
                etr = sa.tile([128, D], F32R, tag="etr")
                nc.vector.tensor_copy(etr[:], tp[:])
                ps = sa_ps.tile([128, G], F32, tag="ps")
                for g in range(3):
                    for k in range(KD):
                        nc.tensor.matmul(
                            ps[:, g * 512 : (g + 1) * 512],
                            etr[:, k * 128 : (k + 1) * 128],
                            wall[:, k * G + g * 512 : k * G + (g + 1) * 512],
                            start=(k == 0),
                            stop=(k == KD - 1),
                        )
                gxc = sa.tile([128, G], F32, tag="gxc")
                nc.vector.tensor_tensor(out=gxc[:], in0=ps[:], in1=bias_sb[:], op=OP.add)
                nc.sync.dma_start(chunk_3d(gx_d, c, G), gxc[:])

        # ---------------- stage B: recurrent scan ----------------
        with (
            tc.tile_pool(name="gxt", bufs=6) as gxp,
            tc.tile_pool(name="ew", bufs=3) as ew,
            tc.tile_pool(name="hp", bufs=3) as hp,
            tc.tile_pool(name="htp", bufs=3) as htp,
            tc.tile_pool(name="sc_ps", bufs=2, space="PSUM") as sc_ps,
            tc.tile_pool(name="tr_ps", bufs=2, space="PSUM") as tr_ps,
        ):
            h_prev = hp.tile([BL, U], F32, tag="h")
            nc.vector.memset(h_prev[:], 0.0)
            ht_prev = htp.tile([128, KU * BL], F32R, tag="ht")
            z0 = hp.tile([128, KU * BL], F32, tag="z0")
            nc.vector.memset(z0[:], 0.0)
            nc.vector.tensor_copy(ht_prev[:], z0[:])
            for t in range(t_steps):
                gxt = gxp.tile([BL, G], F32, tag="gxt")
                nc.sync.dma_start(gxt[:], gx_d[:, t, :])
                ps = sc_ps.tile([BL, G], F32, tag="ps")
                for g in range(3):
                    for k in range(KU):
                        nc.tensor.matmul(
                            ps[:, g * 512 : (g + 1) * 512],
                            ht_prev[:, k * BL : (k + 1) * BL],
                            uwall[:, k * G + g * 512 : k * G + (g + 1) * 512],
                            start=(k == 0),
                            stop=(k == KU - 1),
                        )
                za = ew.tile([BL, 512], F32, tag="za")
                nc.vector.tensor_tensor(out=za[:], in0=ps[:, 0:512], in1=gxt[:, 0:512], op=OP.add)
                zn = ew.tile([BL, 512], F32, tag="zn")  # 1 - z
                nc.scalar.activation(zn[:], za[:], AF.Sigmoid, scale=-1.0)
                ra = ew.tile([BL, 512], F32, tag="ra")
                nc.vector.tensor_tensor(out=ra[:], in0=ps[:, 512:1024], in1=gxt[:, 512:1024], op=OP.add)
                rr = ew.tile([BL, 512], F32, tag="rr")
                nc.scalar.activation(rr[:], ra[:], AF.Sigmoid)
                zm = ew.tile([BL, 512], F32, tag="zm")  # (1-z)*mask
                nc.vector.tensor_scalar_mul(zm[:], zn[:], m_all[:, t : t + 1])
                tmp = ew.tile([BL, 512], F32, tag="tmp")  # r * hh
                nc.vector.tensor_tensor(out=tmp[:], in0=rr[:], in1=ps[:, 1024:1536], op=OP.mult)
                hca = ew.tile([BL, 512], F32, tag="hca")
                nc.vector.tensor_tensor(out=hca[:], in0=tmp[:], in1=gxt[:, 1024:1536], op=OP.add)
                hc = ew.tile([BL, 512], F32, tag="hc")
                nc.scalar.activation(hc[:], hca[:], AF.Tanh)
                e1 = ew.tile([BL, 512], F32, tag="e1")
                nc.vector.tensor_tensor(out=e1[:], in0=hc[:], in1=h_prev[:], op=OP.subtract)
                f1 = ew.tile([BL, 512], F32, tag="f1")
                nc.vector.tensor_tensor(out=f1[:], in0=e1[:], in1=zm[:], op=OP.mult)
                h_new = hp.tile([BL, U], F32, tag="h")
                nc.vector.tensor_tensor(out=h_new[:], in0=h_prev[:], in1=f1[:], op=OP.add)
                nc.sync.dma_start(raw_d[:, t, :], h_new[:])
                tps = tr_ps.tile([128, KU * BL], F32, tag="tps")
                for k in range(KU):
                    nc.tensor.transpose(
                        tps[:, k * BL : (k + 1) * BL],
                        h_new[:, k * 128 : (k + 1) * 128],
                        id_sb[:BL, :BL],
                    )
                ht_new = htp.tile([128, KU * BL], F32R, tag="ht")
                nc.vector.tensor_copy(ht_new[:], tps[:])
                h_prev, ht_prev = h_new, ht_new
            # state output = h at last step
            nc.sync.dma_start(state_d[:], h_prev[:])

        # ---------------- stage C: LayerNorm ----------------
        with tc.tile_pool(name="sc", bufs=3) as sc:
            for c in range(nchunk):
                rawc = sc.tile([128, U], F32, tag="rawc")
                nc.sync.dma_start(rawc[:], chunk_3d(raw_d, c, U))
                s1 = sc.tile([128, 1], F32, tag="s1")
                nc.vector.reduce_sum(out=s1[:], in_=rawc[:], axis=mybir.AxisListType.X)
                sq = sc.tile([128, U], F32, tag="sq")
                nc.scalar.activation(sq[:], rawc[:], AF.Square)
                st = sc.tile([128, 2], F32, tag="st")
                nc.vector.reduce_sum(out=st[:, 1:2], in_=sq[:], axis=mybir.AxisListType.X)
                nc.vector.tensor_copy(st[:, 0:1], s1[:])
                ixc = sc.tile([128, 1], I32, tag="ixc")
                nc.sync.dma_start(ixc[:], scat_d[c * 128 : (c + 1) * 128, :])
                nc.gpsimd.indirect_dma_start(
                    out=stats_d[:],
                    out_offset=bass.IndirectOffsetOnAxis(ap=ixc[:, 0:1], axis=0),
                    in_=st[:], in_offset=None,
                )
            nc.gpsimd.collective_compute(
                "AllReduce", OP.add,
                replica_groups=[[0, 1], [2, 3], [4, 5], [6, 7]],
                ins=[stats_d[:]], outs=[statsr_d[:]],
            )
            for c in range(nchunk):
                ixc = sc.tile([128, 1], I32, tag="ixc")
                nc.sync.dma_start(ixc[:], scat_d[c * 128 : (c + 1) * 128, :])
                cstt = sc.tile([128, 2], F32, tag="cstt")
                nc.gpsimd.indirect_dma_start(
                    out=cstt[:], out_offset=None, in_=statsr_d[:],
                    in_offset=bass.IndirectOffsetOnAxis(ap=ixc[:, 0:1], axis=0),
                )
                mu = sc.tile([128, 1], F32, tag="mu")
                nc.scalar.mul(mu[:], cstt[:, 0:1], 1.0 / (2 * U))
                mq = sc.tile([128, 1], F32, tag="mq")
                nc.scalar.mul(mq[:], cstt[:, 1:2], 1.0 / (2 * U))
                mu2 = sc.tile([128, 1], F32, tag="mu2")
                nc.vector.tensor_tensor(out=mu2[:], in0=mu[:], in1=mu[:], op=OP.mult)
                var = sc.tile([128, 1], F32, tag="var")
                nc.vector.tensor_tensor(out=var[:], in0=mq[:], in1=mu2[:], op=OP.subtract)
                vare = sc.tile([128, 1], F32, tag="vare")
                nc.vector.tensor_scalar_add(vare[:], var[:], LN_EPS)
                rec = sc.tile([128, 1], F32, tag="rec")
                nc.vector.reciprocal(rec[:], vare[:])
                rstd = sc.tile([128, 1], F32, tag="rstd")
                nc.scalar.activation(rstd[:], rec[:], AF.Sqrt)
                rawc = sc.tile([128, U], F32, tag="rawc")
                nc.sync.dma_start(rawc[:], chunk_3d(raw_d, c, U))
                y = sc.tile([128, U], F32, tag="y")
                nc.vector.tensor_scalar(y[:], rawc[:], mu[:, 0:1], rstd[:, 0:1], OP.subtract, OP.mult)
                yg = sc.tile([128, U], F32, tag="yg")
                nc.vector.tensor_tensor(out=yg[:], in0=y[:], in1=gam_sb[:], op=OP.mult)
                yb = sc.tile([128, U], F32, tag="yb")
                nc.vector.tensor_tensor(out=yb[:], in0=yg[:], in1=bet_sb[:], op=OP.add)
                nc.sync.dma_start(outh_d[c * 128 : (c + 1) * 128, :], yb[:])
    nc.compile()
    return nc


def _prep_inputs(x, emb, W_fwd, U_fwd, b_fwd, W_bwd, U_bwd, b_bwd, gamma, beta, t_steps=T):
    f32 = np.float32
    in_maps = []
    for core in range(8):
        p, d = core // 2, core % 2
        xb = np.asarray(x[p * BL : (p + 1) * BL, :t_steps], dtype=np.int32)
        if d == 1:
            xb = xb[:, ::-1]
        Wd, Ud, bd = (W_fwd, U_fwd, b_fwd) if d == 0 else (W_bwd, U_bwd, b_bwd)
        bias = np.asarray(bd[0], f32).copy()
        bias[:1024] += np.asarray(bd[1][:1024], f32)
        gh = np.asarray(gamma[d * U : (d + 1) * U], f32)
        bh = np.asarray(beta[d * U : (d + 1) * U], f32)
        tloc = np.arange(t_steps)
        tabs = tloc if d == 0 else (t_steps - 1 - tloc)
        scat = (np.arange(BL)[:, None] * t_steps + tabs[None, :]).reshape(-1, 1)
        in_maps.append({
            "x": np.ascontiguousarray(xb).reshape(-1, 1),
            "emb": np.asarray(emb, f32),
            "w": np.asarray(Wd, f32),
            "uw": np.asarray(Ud, f32),
            "bias": np.tile(bias[None, :], (128, 1)),
            "gam": np.tile(gh[None, :], (128, 1)),
            "bet": np.tile(bh[None, :], (128, 1)),
            "scat": scat.astype(np.int32),
        })
    return in_maps


def _assemble(results, t_steps=T):
    out = np.empty((B, t_steps, 2 * U), np.float32)
    state = np.empty((B, 2 * U), np.float32)
    for p in range(4):
        rf, rb = results[2 * p], results[2 * p + 1]
        sl = slice(p * BL, (p + 1) * BL)
        out[sl, :, :U] = rf["outh"].reshape(BL, t_steps, U)
        out[sl, :, U:] = rb["outh"].reshape(BL, t_steps, U)[:, ::-1, :]
        state[sl, :U] = rf["state"]
        state[sl, U:] = rb["state"]
    return out, state


def run(inputs, t_steps=T):
    if t_steps not in _cache:
        _cache[t_steps] = _build(t_steps)
    nc = _cache[t_steps]
    in_maps = _prep_inputs(**inputs, t_steps=t_steps)
    res = run_bass_kernel_spmd(nc, in_maps, list(range(8)))
    return _assemble(res.results, t_steps)


def kernel(x, emb, W_fwd, U_fwd, b_fwd, W_bwd, U_bwd, b_bwd, gamma, beta):
    out, state = run(dict(x=x, emb=emb, W_fwd=W_fwd, U_fwd=U_fwd, b_fwd=b_fwd,
                          W_bwd=W_bwd, U_bwd=U_bwd, b_bwd=b_bwd,
                          gamma=gamma, beta=beta))
    return out, state
